# revision 1
# baseline (speedup 1.0000x reference)
"""CosFormer causal attention — Trainium2 Bass kernel, 8 NeuronCores.

Sharding: core i = (batch b = i//4, head-group g = i%4 covering heads 2g, 2g+1).
Each core computes the qkv projection for its two heads, chunked causal linear
attention (cos/sin feature channels), and a partial output projection over its
128 context channels. The host unshards by summing the 4 per-core partials of
each batch (the output projection's contraction is sharded over heads) and
adding b_out.

Key layout/perf choices:
- Per-head q/k features in [feat, t] layout as one [128, T] float32r tile
  (rows 0:64 = relu(.)*cos_t, 64:128 = relu(.)*sin_t), produced by projecting
  with duplicated weight columns (PE cost scales with the moving dim, not M).
- float32r everywhere on the PE: 1 cycle/row when the moving dim is >= 256
  (vs 4 for plain fp32), fp32 PSUM accumulation.
- Attention runs in 256-wide query super-chunks: two 128-wide key stripes are
  scored against the full 256 query band (moving dim 256) and masked, so every
  attention matmul streams at full rate. A [2d, d+2] state carries the prefix
  between super-chunks.
- Normalization: norm row -> PE-transpose to a [t, 1] column -> 1-elem/lane
  reciprocal -> applied per-partition to the per-head out-projection partials.

Fully self-contained: hardcodes B=2, T=1024, E=512, H=8.
"""

import math
from contextlib import ExitStack

import numpy as np

import concourse.bass as bass
import concourse.mybir as mybir
import concourse.tile as tile
from concourse.bass_utils import run_bass_kernel_spmd
from concourse.vector_clock import ScopedClock

B, T, E = 2, 1024, 512
H, D = 8, 64
S = 128            # key stripe size
SC = 256           # query super-chunk size
NSC = T // SC      # 4
F32 = mybir.dt.float32
F32R = mybir.dt.float32r
EPS = 1e-6


def _install_drain_patch():
    """This walrus build rejects a Drain carrying >1 sem wait. Split the
    Tile-exit drain's waits across single-wait SP nops."""
    if getattr(tile.TileContext, "_drain_patch_installed", False):
        return

    def _patched(self, tick_clock, wait_clock):
        nc = self.nc
        pre = nc.sync.nop(nofuse=True)
        wait_clock.add_sem_waits(pre.ins, ScopedClock({None: tick_clock.global_clock}))
        waits = list(pre.ins.sync_info.on_wait or []) if pre.ins.sync_info else []
        if len(waits) > 1:
            pre.ins.sync_info.on_wait = waits[:1]
            for w in waits[1:]:
                n = nc.sync.nop(nofuse=True)
                if n.ins.sync_info is None:
                    n.ins.sync_info = mybir.SyncInfo(on_wait=[w], on_update=[])
                else:
                    n.ins.sync_info.on_wait = [w]
        nc.sync.drain()
        nc.all_engine_barrier()
        popped = nc._tile_sem_poison_stack.pop()
        assert popped is self._sem_poison

    tile.TileContext._drain_and_barrier = _patched
    tile.TileContext._drain_patch_installed = True


def _split_multi_waits(nc):
    """This walrus build only codegens ONE sync-wait command per instruction.
    Move excess waits onto same-engine NoOps inserted just before."""
    ctr = [0]

    def _mk_nop(engine, wait):
        ctr[0] += 1
        return mybir.InstNoOp(
            name=f"I-waitnop{ctr[0]}",
            engine=engine,
            ins=[],
            outs=[],
            sync_info=mybir.SyncInfo(on_wait=[wait], on_update=[]),
        )

    for f in nc.m.functions:
        for bb in f.blocks:
            new_insts = []
            for inst in bb.instructions:
                si = inst.sync_info
                waits = list(si.on_wait) if si and si.on_wait else []
                if len(waits) > 1:
                    for w in waits[:-1]:
                        new_insts.append(_mk_nop(inst.engine, w))
                    si.on_wait = waits[-1:]
                new_insts.append(inst)
            bb.instructions[:] = new_insts


def build_program() -> bass.Bass:
    _install_drain_patch()
    nc = bass.Bass()

    # wqkf: duplicated weight cols [qf_h0 | qf_h1 | kf_h0 | kf_h1], each 128 wide
    xt = nc.declare_dram_parameter("xt", [E, T], F32R, isOutput=False)        # x[b].T
    wqkf = nc.declare_dram_parameter("wqkf", [E, 512], F32R, isOutput=False)
    wvt = nc.declare_dram_parameter("wvt", [E, 128], F32R, isOutput=False)    # [v0 v1].T
    bqkf = nc.declare_dram_parameter("bqkf", [640], F32, isOutput=False)      # dup'd qk biases + v bias
    csrep = nc.declare_dram_parameter("csrep", [128, T], F32, isOutput=False)  # [cos;sin]
    w2 = nc.declare_dram_parameter("w2", [128, E], F32R, isOutput=False)
    identin = nc.declare_dram_parameter("identin", [128, 128], F32R, isOutput=False)
    m0in = nc.declare_dram_parameter("m0in", [S, SC], F32, isOutput=False)    # [tri | ones]
    out = nc.declare_dram_parameter("out", [T, E], F32, isOutput=True)

    with tile.TileContext(nc) as tc, ExitStack() as ctx:
        singles = ctx.enter_context(tc.tile_pool(name="singles", bufs=1))
        kf_pool = ctx.enter_context(tc.tile_pool(name="kf", bufs=4))
        atm_pool = ctx.enter_context(tc.tile_pool(name="atm", bufs=3))
        osb_pool = ctx.enter_context(tc.tile_pool(name="osb", bufs=2))
        nrm_pool = ctx.enter_context(tc.tile_pool(name="nrm", bufs=4))
        pp_big = ctx.enter_context(tc.tile_pool(name="pp_big", bufs=2, space="PSUM"))
        pp_mm = ctx.enter_context(tc.tile_pool(name="pp_mm", bufs=2, space="PSUM"))
        pp_kt = ctx.enter_context(tc.tile_pool(name="pp_kt", bufs=2, space="PSUM"))
        pp_cs = ctx.enter_context(tc.tile_pool(name="pp_cs", bufs=2, space="PSUM"))

        # ---- constant / input tiles -------------------------------------
        # critical path first on the sync (HWDGE) queue: wqkf/xt per k-block;
        # everything else trickles in on the gpsimd (SWDGE) queue.
        xt_s = singles.tile([128, 4, T], F32R)
        xt_r = xt.rearrange("(kk p) t -> p kk t", p=128)
        wqkf_s = singles.tile([128, 4, 512], F32R)
        wqkf_r = wqkf.rearrange("(kk p) c -> p kk c", p=128)
        for kk in range(4):
            nc.sync.dma_start(out=wqkf_s[:, kk, :], in_=wqkf_r[:, kk, :])
            nc.sync.dma_start(out=xt_s[:, kk, :], in_=xt_r[:, kk, :])
        wvt_s = singles.tile([128, 4, 128], F32R)
        nc.sync.dma_start(out=wvt_s, in_=wvt.rearrange("(kk p) c -> p kk c", p=128))
        biases = []
        for bi in range(4):
            t_ = singles.tile([128, 1], F32, name=f"bias{bi}")
            nc.gpsimd.dma_start(out=t_, in_=bqkf[bi * 128:(bi + 1) * 128, None])
            biases.append(t_)
        bias_v = singles.tile([128, 1], F32, name="bias_v")
        nc.gpsimd.dma_start(out=bias_v, in_=bqkf[512:640, None])
        cs_s = singles.tile([128, T], F32)
        nc.gpsimd.dma_start(out=cs_s, in_=csrep[:, :])
        w2h = []
        for h in range(2):
            t_ = singles.tile([D, E], F32R, name=f"w2h{h}")
            nc.gpsimd.dma_start(out=t_, in_=w2[h * D:(h + 1) * D, :])
            w2h.append(t_)
        ident = singles.tile([128, 128], F32R)
        nc.gpsimd.dma_start(out=ident, in_=identin[:, :])
        m0_s = singles.tile([S, SC], F32)
        nc.gpsimd.dma_start(out=m0_s, in_=m0in[:, :])
        eps_t = singles.tile([1, 1], F32, name="eps_t")
        nc.vector.memset(eps_t, EPS)
        onesz_col = singles.tile([128, 2], F32, name="onesz_col")
        nc.vector.memset(onesz_col[:, 0:1], 1.0)
        nc.vector.memset(onesz_col[:, 1:2], 0.0)

        # per-head stacked feature tiles [cos;sin] x t
        qfT = [singles.tile([128, T], F32R, name=f"qfT{h}") for h in range(2)]
        kfT = [singles.tile([128, T], F32R, name=f"kfT{h}") for h in range(2)]
        vT = singles.tile([128, T], F32R, name="vT")
        state = [singles.tile([128, D + 2], F32R, name=f"state{h}") for h in range(2)]
        # persistent V' ring: [head][stripe], ones/pad cols written once
        vp_ring = [[singles.tile([S, D + 2], F32R, name=f"vpr{h}_{ci}")
                    for ci in range(2)] for h in range(2)]
        for h in range(2):
            for ci in range(2):
                nc.scalar.copy(vp_ring[h][ci][:, D:D + 2], onesz_col)

        # ---- q/k/v features, first t-half fully before the second --------
        # block bi: 0=qf_h0, 1=qf_h1, 2=kf_h0, 3=kf_h1
        for th in range(2):
            tslh = slice(th * 512, (th + 1) * 512)
            for bi, dst in ((0, qfT[0]), (1, qfT[1]), (2, kfT[0]), (3, kfT[1])):
                ps = pp_big.tile([128, 512], F32, tag="big", name=f"psB{bi}_{th}")
                for kk in range(4):
                    nc.tensor.matmul(
                        ps,
                        wqkf_s[:, kk, bi * 128:(bi + 1) * 128],
                        xt_s[:, kk, tslh],
                        start=(kk == 0),
                        stop=(kk == 3),
                    )
                nc.scalar.activation(
                    out=dst[:, tslh],
                    in_=ps,
                    func=mybir.ActivationFunctionType.Relu,
                    bias=biases[bi],
                    scale=1.0,
                )
                nc.vector.tensor_mul(dst[:, tslh], dst[:, tslh], cs_s[:, tslh])
            ps = pp_big.tile([128, 512], F32, tag="big", name=f"psV{th}")
            for kk in range(4):
                nc.tensor.matmul(
                    ps,
                    wvt_s[:, kk, :],
                    xt_s[:, kk, tslh],
                    start=(kk == 0),
                    stop=(kk == 3),
                )
            nc.scalar.activation(
                out=vT[:, tslh],
                in_=ps,
                func=mybir.ActivationFunctionType.Identity,
                bias=bias_v,
                scale=1.0,
            )

        # ---- attention, 256-wide query super-chunks ----------------------
        for sc in range(NSC):
            t0 = sc * SC
            band = slice(t0, t0 + SC)
            sub = [slice(t0, t0 + S), slice(t0 + S, t0 + 2 * S)]

            # stripe transposes: kfT/vT [*, t] -> [t, *] per 128-stripe
            kfeat = [[None, None], [None, None]]  # [ci][h]
            vp = [[None, None], [None, None]]     # [ci][h]
            kt_tiles = []
            for ci in range(2):
                ps_kt = pp_kt.tile([128, 392], F32R, tag="kt", name=f"pskt{sc}_{ci}")
                kt_tiles.append(ps_kt)
                for h in range(2):
                    kfeat[ci][h] = kf_pool.tile(
                        [S, 128], F32R, tag=f"kf{h}", name=f"kfeat{sc}_{ci}_{h}")
                    nc.tensor.transpose(
                        ps_kt[:, h * 128:(h + 1) * 128], kfT[h][:, sub[ci]], ident)
                nc.vector.tensor_copy(kfeat[ci][0], ps_kt[:, 0:128])
                nc.scalar.copy(kfeat[ci][1], ps_kt[:, 128:256])
                nc.tensor.transpose(ps_kt[:, 256:384], vT[:, sub[ci]], ident)
                for h in range(2):
                    vp[ci][h] = vp_ring[h][ci]
                nc.vector.tensor_copy(vp[ci][0][:, 0:D], ps_kt[:, 256:256 + D])
                nc.scalar.copy(vp[ci][1][:, 0:D], ps_kt[:, 256 + D:256 + 2 * D])

            ps_o = [[None, None], [None, None]]   # [ci][h]
            ncol = [[None, None], [None, None]]   # [ci][h]
            for h in range(2):
                # stripe 0 scores the whole band; stripe 1 only its own half
                ps_a0 = pp_mm.tile([S, SC], F32, tag="mm", name=f"psa{sc}_0_{h}")
                nc.tensor.matmul(ps_a0, kfT[h][:, sub[0]], qfT[h][:, band],
                                 start=True, stop=True)
                atm0 = atm_pool.tile([S, SC], F32R, tag="atm", name=f"atm{sc}_0_{h}")
                nc.vector.tensor_mul(atm0, ps_a0, m0_s)
                ps_a1 = pp_mm.tile([S, S], F32, tag="mm", name=f"psa{sc}_1_{h}")
                nc.tensor.matmul(ps_a1, kfT[h][:, sub[1]], qfT[h][:, sub[1]],
                                 start=True, stop=True)
                atm1 = atm_pool.tile([S, S], F32R, tag="atm1", name=f"atm{sc}_1_{h}")
                nc.vector.tensor_mul(atm1, ps_a1, m0_s[:, 0:S])

                # ctx^T (+norm row 64) = prefix-state inter + two stripe intras
                ps_c = pp_cs.tile([D + 2, SC], F32, tag="cs", name=f"psc{sc}_{h}")
                if sc > 0:
                    nc.tensor.matmul(ps_c, state[h], qfT[h][:, band], start=True, stop=False)
                    nc.tensor.matmul(ps_c, vp[0][h], atm0, start=False, stop=False)
                    nc.tensor.matmul(ps_c[:, S:SC], vp[1][h], atm1, start=False, stop=True)
                else:
                    nc.tensor.matmul(ps_c, vp[0][h], atm0, start=True, stop=False)
                    nc.tensor.matmul(ps_c[:, S:SC], vp[1][h], atm1, start=False, stop=True)

                # state += Kf^T V' over both stripes
                ps_s = pp_cs.tile([128, D + 2], F32, tag="cs", name=f"pss{sc}_{h}")
                nc.tensor.matmul(ps_s, kfeat[0][h], vp[0][h], start=True, stop=False)
                nc.tensor.matmul(ps_s, kfeat[1][h], vp[1][h], start=False, stop=True)
                if sc == 0:
                    nc.vector.tensor_copy(state[h], ps_s)
                else:
                    nc.vector.tensor_add(state[h], state[h], ps_s)

                # norm row -> [t,1] columns (PE transpose) -> reciprocal
                nrow = nrm_pool.tile([1, SC], F32R, tag="nrow", name=f"nrow{sc}_{h}")
                nc.scalar.activation(out=nrow, in_=ps_c[D:D + 1, :],
                                     func=mybir.ActivationFunctionType.Identity,
                                     bias=eps_t[0:1, 0:1], scale=1.0)
                for ci in range(2):
                    ps_n = kt_tiles[ci][:, 384 + 2 * h:386 + 2 * h]
                    nc.tensor.transpose(ps_n, nrow[:, ci * S:(ci + 1) * S], ident[0:1, 0:2])
                    nc_t = nrm_pool.tile([S, 1], F32, tag="ncol", name=f"ncol{sc}_{ci}_{h}")
                    nc.vector.reciprocal(nc_t, ps_n[:, 0:1])
                    ncol[ci][h] = nc_t

                # unnormalized ctx -> SBUF; per-stripe per-head out-projection
                ctxu = nrm_pool.tile([D, SC], F32R, tag="ctxu", name=f"ctxu{sc}_{h}")
                nc.scalar.copy(ctxu, ps_c[0:D, :])
                for ci in range(2):
                    ps = pp_big.tile([128, E], F32, tag="big", name=f"pso{sc}_{ci}_{h}")
                    nc.tensor.matmul(ps, ctxu[:, ci * S:(ci + 1) * S], w2h[h],
                                     start=True, stop=True)
                    ps_o[ci][h] = ps

            # scale by 1/norm (per-partition) and combine heads
            for ci in range(2):
                o_s = osb_pool.tile([128, E], F32, tag="osb", name=f"os{sc}_{ci}")
                nc.scalar.activation(out=o_s, in_=ps_o[ci][0],
                                     func=mybir.ActivationFunctionType.Copy,
                                     scale=ncol[ci][0])
                nc.vector.scalar_tensor_tensor(
                    out=o_s, in0=ps_o[ci][1], scalar=ncol[ci][1], in1=o_s,
                    op0=mybir.AluOpType.mult, op1=mybir.AluOpType.add,
                )
                nc.sync.dma_start(out=out[sub[ci], :], in_=o_s)

    _split_multi_waits(nc)
    return nc


_PROGRAM = None


def _get_program():
    global _PROGRAM
    if _PROGRAM is None:
        _PROGRAM = build_program()
    return _PROGRAM


def _make_in_maps(x, w_qkv, b_qkv, w_out):
    pos = np.arange(T, dtype=np.float32)
    ang = (math.pi / 2) * pos / T
    cosw = np.cos(ang).astype(np.float32)
    sinw = np.sin(ang).astype(np.float32)
    csrep = np.concatenate([
        np.broadcast_to(cosw[None, :], (D, T)),
        np.broadcast_to(sinw[None, :], (D, T)),
    ], 0).astype(np.float32)
    tri = np.triu(np.ones((S, S), np.float32))
    m0 = np.concatenate([tri, np.ones((S, S), np.float32)], 1)

    in_maps = []
    for i in range(8):
        b, g = divmod(i, 4)
        h0, h1 = 2 * g, 2 * g + 1
        wq = lambda h: w_qkv[h * D:(h + 1) * D]
        wk = lambda h: w_qkv[E + h * D:E + (h + 1) * D]
        wv = lambda h: w_qkv[2 * E + h * D:2 * E + (h + 1) * D]
        bq = lambda h: b_qkv[h * D:(h + 1) * D]
        bk = lambda h: b_qkv[E + h * D:E + (h + 1) * D]
        bv = lambda h: b_qkv[2 * E + h * D:2 * E + (h + 1) * D]
        hcols = np.r_[h0 * D:(h0 + 1) * D, h1 * D:(h1 + 1) * D]
        wqkf = np.concatenate([
            wq(h0), wq(h0), wq(h1), wq(h1), wk(h0), wk(h0), wk(h1), wk(h1)
        ], 0).T
        bqkf = np.concatenate([
            bq(h0), bq(h0), bq(h1), bq(h1), bk(h0), bk(h0), bk(h1), bk(h1),
            bv(h0), bv(h1)
        ])
        in_maps.append({
            "xt": np.ascontiguousarray(x[b].T),
            "wqkf": np.ascontiguousarray(wqkf),
            "wvt": np.ascontiguousarray(np.concatenate([wv(h0), wv(h1)], 0).T),
            "bqkf": np.ascontiguousarray(bqkf),
            "csrep": csrep,
            "w2": np.ascontiguousarray(w_out[:, hcols].T),
            "identin": np.eye(128, dtype=np.float32),
            "m0in": m0,
        })
    return in_maps


def run(inputs, trace=False):
    x = np.asarray(inputs["x"], dtype=np.float32)
    w_qkv = np.asarray(inputs["w_qkv"], dtype=np.float32)
    b_qkv = np.asarray(inputs["b_qkv"], dtype=np.float32)
    w_out = np.asarray(inputs["w_out"], dtype=np.float32)
    b_out = np.asarray(inputs["b_out"], dtype=np.float32)

    nc = _get_program()
    in_maps = _make_in_maps(x, w_qkv, b_qkv, w_out)
    res = run_bass_kernel_spmd(nc, in_maps, list(range(8)), trace=trace)

    out = np.empty((B, T, E), dtype=np.float32)
    for b in range(B):
        acc = res.results[4 * b]["out"].astype(np.float32)
        for g in range(1, 4):
            acc = acc + res.results[4 * b + g]["out"]
        out[b] = acc + b_out[None, :]
    return out, res


def kernel(**inputs) -> np.ndarray:
    out, _ = run(inputs, trace=False)
    return out



# revision 10
# speedup vs baseline: 1.1294x; 1.1294x over previous
"""CosFormer causal attention — Trainium2 Bass kernel, 8 NeuronCores.

Sharding: core i = (batch b = i//4, head-group g = i%4 covering heads 2g, 2g+1).
Each core computes the qkv projection for its two heads, chunked causal linear
attention (cosFormer cos/sin features), and a partial output projection over
its 128 context channels. The host sums the 4 per-core partials per batch and
adds b_out.

v2 layout/perf choices (vs v1):
- bf16 operands on every matmul: 1 cycle/row at any moving size (f32r needs
  moving>=256 for that) and 4x cheaper LDWEIGHTS.
- Deduplicated qkv projection (24 matmuls instead of 40): scores contract the
  raw 64-dim relu features; the cos/sin positional factors are folded into a
  host-precomputed causal mask m[sp,tq] = (sp<=tq)*cos((sp-tq)*pi/(2T)) since
  within a chunk they only depend on s-t. Only the q side needs the stacked
  [cos*q'; sin*q'] form (moving operand of the carried-state matmul); the k
  side needs it only in transposed [s, feat] form, built from one per-stripe
  transpose of raw k' scaled per-partition by cos_s/sin_s columns.
- DMA descriptor-issue (~0.65us per 128-row DMA, serialized per engine) is
  minimized by packing all inputs into 5 wide host-prepped tensors, issued
  from the sync/scalar/gpsimd queues in parallel.
- PSUM packing: 4 pools x 2 bufs = 8 banks, with scores0+scores1, rkT+vT+
  normT, and ps_c+ps_s sharing banks.

Fully self-contained: hardcodes B=2, T=1024, E=512, H=8.
"""

import math
from contextlib import ExitStack

import numpy as np
import ml_dtypes

import concourse.bass as bass
import concourse.mybir as mybir
import concourse.tile as tile
from concourse.bass_utils import run_bass_kernel_spmd
from concourse.vector_clock import ScopedClock

B, T, E = 2, 1024, 512
H, D = 8, 64
S = 128            # key stripe size
SC = 256           # query super-chunk size
NSC = T // SC      # 4
F32 = mybir.dt.float32
BF16 = mybir.dt.bfloat16
EPS = 1e-6

# combo2 column offsets
C2_WV = 0          # [kk*128] v-weight blocks, 512 cols
C2_CS = 512        # [cos;sin] x t, 1024 cols
C2_CSW = 1536      # [sin;cos] x t, 1024 cols
C2_M0 = 2560       # causal cos mask, 256 cols
C2_W2 = 2816       # w2 pack (h0 rows 0:64, h1 rows 64:128), 512 cols
C2_TOT = 3328


def _install_drain_patch():
    """This walrus build rejects a Drain carrying >1 sem wait. Split the
    Tile-exit drain's waits across single-wait SP nops."""
    if getattr(tile.TileContext, "_drain_patch_installed", False):
        return

    def _patched(self, tick_clock, wait_clock):
        nc = self.nc
        pre = nc.sync.nop(nofuse=True)
        wait_clock.add_sem_waits(pre.ins, ScopedClock({None: tick_clock.global_clock}))
        waits = list(pre.ins.sync_info.on_wait or []) if pre.ins.sync_info else []
        if len(waits) > 1:
            pre.ins.sync_info.on_wait = waits[:1]
            for w in waits[1:]:
                n = nc.sync.nop(nofuse=True)
                if n.ins.sync_info is None:
                    n.ins.sync_info = mybir.SyncInfo(on_wait=[w], on_update=[])
                else:
                    n.ins.sync_info.on_wait = [w]
        nc.sync.drain()
        nc.all_engine_barrier()
        popped = nc._tile_sem_poison_stack.pop()
        assert popped is self._sem_poison

    tile.TileContext._drain_and_barrier = _patched
    tile.TileContext._drain_patch_installed = True


def _split_multi_waits(nc):
    """This walrus build only codegens ONE sync-wait command per instruction.
    Move excess waits onto same-engine NoOps inserted just before."""
    ctr = [0]

    def _mk_nop(engine, wait):
        ctr[0] += 1
        return mybir.InstNoOp(
            name=f"I-waitnop{ctr[0]}",
            engine=engine,
            ins=[],
            outs=[],
            sync_info=mybir.SyncInfo(on_wait=[wait], on_update=[]),
        )

    for f in nc.m.functions:
        for bb in f.blocks:
            new_insts = []
            for inst in bb.instructions:
                si = inst.sync_info
                waits = list(si.on_wait) if si and si.on_wait else []
                if len(waits) > 1:
                    for w in waits[:-1]:
                        new_insts.append(_mk_nop(inst.engine, w))
                    si.on_wait = waits[-1:]
                new_insts.append(inst)
            bb.instructions[:] = new_insts


def build_program() -> bass.Bass:
    _install_drain_patch()
    nc = bass.Bass()

    # host-packed inputs ([p, ...] layouts, contiguous per partition row)
    xtp = nc.declare_dram_parameter("xtp", [128, 4 * T], BF16, isOutput=False)
    wqkk = nc.declare_dram_parameter("wqkk", [128, 1024], BF16, isOutput=False)
    combo1 = nc.declare_dram_parameter("combo1", [128, 148], BF16, isOutput=False)
    combo2 = nc.declare_dram_parameter("combo2", [128, C2_TOT], BF16, isOutput=False)
    out = nc.declare_dram_parameter("out", [T, E], BF16, isOutput=True)

    with tile.TileContext(nc) as tc, ExitStack() as ctx:
        singles = ctx.enter_context(tc.tile_pool(name="singles", bufs=1))
        kf_pool = ctx.enter_context(tc.tile_pool(name="kf", bufs=4))
        atm_pool = ctx.enter_context(tc.tile_pool(name="atm", bufs=2))
        osb_pool = ctx.enter_context(tc.tile_pool(name="osb", bufs=2))
        nrm_pool = ctx.enter_context(tc.tile_pool(name="nrm", bufs=2))
        pp_big = ctx.enter_context(tc.tile_pool(name="pp_big", bufs=2, space="PSUM"))
        pp_kt = ctx.enter_context(tc.tile_pool(name="pp_kt", bufs=2, space="PSUM"))
        pp_mm = ctx.enter_context(tc.tile_pool(name="pp_mm", bufs=2, space="PSUM"))
        pp_cs = ctx.enter_context(tc.tile_pool(name="pp_cs", bufs=2, space="PSUM"))

        # ---- input DMAs: 5 wide transfers over 3 parallel issue queues ----
        wqkk_s = singles.tile([128, 1024], BF16)
        nc.sync.dma_start(out=wqkk_s, in_=wqkk[:, :])
        # xtp host layout: [p, th, kk, 512] so each th-half is one contiguous
        # 4KB-per-partition transfer
        xt_s = singles.tile([128, 2, 4, 512], BF16)
        xt_r = xtp.rearrange("p (th kk t) -> p th kk t", th=2, kk=4)
        nc.sync.dma_start(out=xt_s[:, 0], in_=xt_r[:, 0])
        nc.scalar.dma_start(out=xt_s[:, 1], in_=xt_r[:, 1])
        c1_s = singles.tile([128, 148], BF16)
        nc.gpsimd.dma_start(out=c1_s, in_=combo1[:, :])
        c2_s = singles.tile([128, C2_TOT], BF16)
        nc.gpsimd.dma_start(out=c2_s, in_=combo2[:, :])

        ident = c1_s[:, 0:128]
        bcol = singles.tile([128, 4], F32, name="bcol_f")
        nc.scalar.copy(bcol, c1_s[:, 128:132])
        cscol = singles.tile([128, 16], F32, name="cscol_f")
        nc.scalar.copy(cscol, c1_s[:, 132:148])
        cs_s = c2_s[:, C2_CS:C2_CS + T]
        csw_s = c2_s[:, C2_CSW:C2_CSW + T]
        m0_s = c2_s[:, C2_M0:C2_M0 + SC]
        w2p = c2_s[:, C2_W2:C2_W2 + E]

        eps_t = singles.tile([1, 1], F32, name="eps_t")
        nc.vector.memset(eps_t, EPS)
        onesz_col = singles.tile([128, 2], BF16, name="onesz_col")
        nc.vector.memset(onesz_col[:, 0:1], 1.0)
        nc.vector.memset(onesz_col[:, 1:2], 0.0)
        ones1 = singles.tile([1, 64], BF16, name="ones1")
        nc.vector.memset(ones1, 1.0)

        # persistent feature tiles
        r_q = singles.tile([128, T], BF16, name="r_q")   # [q'_h0; q'_h1] x t
        r_k = singles.tile([128, T], BF16, name="r_k")
        vT = singles.tile([128, T], BF16, name="vT")
        qfs = [singles.tile([128, T], BF16, name=f"qfs{h}") for h in range(2)]
        state_f = [singles.tile([128, D + 2], F32, name=f"statef{h}") for h in range(2)]
        state_b = [singles.tile([128, D + 2], BF16, name=f"stateb{h}") for h in range(2)]
        vp_ring = [[singles.tile([S, D + 2], BF16, name=f"vpr{h}_{ci}")
                    for ci in range(2)] for h in range(2)]
        for h in range(2):
            for ci in range(2):
                nc.scalar.copy(vp_ring[h][ci][:, D:D + 2], onesz_col)

        # ---- qkv projection, dedup'd, th halves -------------------------
        for th in range(2):
            tslh = slice(th * 512, (th + 1) * 512)
            for bidx, dst, func in (
                (0, r_q, mybir.ActivationFunctionType.Relu),
                (1, r_k, mybir.ActivationFunctionType.Relu),
                (2, vT, mybir.ActivationFunctionType.Identity),
            ):
                ps = pp_big.tile([128, 512], F32, tag="big", name=f"psB{bidx}_{th}")
                for kk in range(4):
                    if bidx < 2:
                        w_ap = wqkk_s[:, bidx * 512 + kk * 128:bidx * 512 + (kk + 1) * 128]
                    else:
                        w_ap = c2_s[:, C2_WV + kk * 128:C2_WV + (kk + 1) * 128]
                    nc.tensor.matmul(
                        ps, w_ap, xt_s[:, th, kk, :],
                        start=(kk == 0), stop=(kk == 3),
                    )
                nc.scalar.activation(
                    out=dst[:, tslh], in_=ps, func=func,
                    bias=bcol[:, bidx:bidx + 1], scale=1.0,
                )
            # stacked q features: [cos*q'_h; sin*q'_h]
            nc.gpsimd.tensor_mul(qfs[0][0:64, tslh], r_q[0:64, tslh], cs_s[0:64, tslh])
            nc.gpsimd.tensor_mul(qfs[0][64:128, tslh], r_q[0:64, tslh], csw_s[0:64, tslh])
            nc.gpsimd.tensor_mul(qfs[1][0:64, tslh], r_q[64:128, tslh], csw_s[64:128, tslh])
            nc.gpsimd.tensor_mul(qfs[1][64:128, tslh], r_q[64:128, tslh], cs_s[64:128, tslh])

        # ---- attention, 256-wide query super-chunks ----------------------
        for sc in range(NSC):
            t0 = sc * SC
            band = slice(t0, t0 + SC)
            sub = [slice(t0, t0 + S), slice(t0 + S, t0 + 2 * S)]

            # stripe prep: one transpose of raw k' (both heads) + v per stripe
            kfeat = [[None, None], [None, None]]  # [ci][h]
            vp = [[None, None], [None, None]]
            kts = [None, None]
            for ci in range(2):
                si = 2 * sc + ci
                kt = pp_kt.tile([128, 256], BF16, tag="kt", name=f"kt{sc}_{ci}")
                kts[ci] = kt
                nc.tensor.transpose(kt[:, 0:128], r_k[:, sub[ci]], ident)
                nc.tensor.transpose(kt[:, 128:256], vT[:, sub[ci]], ident)
                for h in range(2):
                    kf = kf_pool.tile([S, 128], BF16, tag=f"kf{h}",
                                      name=f"kfeat{sc}_{ci}_{h}")
                    if ci == 0:
                        nc.vector.tensor_scalar_mul(
                            kf[:, 0:64], kt[:, h * 64:(h + 1) * 64],
                            cscol[:, si:si + 1])
                        nc.vector.tensor_scalar_mul(
                            kf[:, 64:128], kt[:, h * 64:(h + 1) * 64],
                            cscol[:, 8 + si:9 + si])
                    else:
                        nc.scalar.activation(
                            out=kf[:, 0:64], in_=kt[:, h * 64:(h + 1) * 64],
                            func=mybir.ActivationFunctionType.Copy,
                            scale=cscol[:, si:si + 1])
                        nc.scalar.activation(
                            out=kf[:, 64:128], in_=kt[:, h * 64:(h + 1) * 64],
                            func=mybir.ActivationFunctionType.Copy,
                            scale=cscol[:, 8 + si:9 + si])
                    kfeat[ci][h] = kf
                    vp[ci][h] = vp_ring[h][ci]
                nc.scalar.copy(vp[ci][0][:, 0:D], kt[:, 128:128 + D])
                nc.scalar.copy(vp[ci][1][:, 0:D], kt[:, 128 + D:128 + 2 * D])

            nrow = [nrm_pool.tile([1, SC], F32, tag=f"nrow{h}", name=f"nrow{sc}_{h}")
                    for h in range(2)]
            rnb = [nrm_pool.tile([1, SC], BF16, tag=f"rnb{h}", name=f"rnb{sc}_{h}")
                   for h in range(2)]
            ctxn = nrm_pool.tile([128, SC], BF16, tag="ctxn", name=f"ctxn{sc}")
            cs_h = [None, None]
            for h in range(2):
                hb = h * 64
                hsl = slice(hb, hb + 64)
                # scores: raw 64-dim features, positional factors in the mask
                mm = pp_mm.tile([128, 384], F32, tag="mm", name=f"mm{sc}_{h}")
                nc.tensor.matmul(mm[:, 0:256], r_k[hsl, sub[0]], r_q[hsl, band],
                                 start=True, stop=True)
                nc.tensor.matmul(mm[:, 256:384], r_k[hsl, sub[1]], r_q[hsl, sub[1]],
                                 start=True, stop=True)
                atm = atm_pool.tile([S, 384], BF16, tag="atm", name=f"atm{sc}_{h}")
                nc.vector.tensor_mul(atm[:, 0:256], mm[:, 0:256], m0_s)
                nc.vector.tensor_mul(atm[:, 256:384], mm[:, 256:384], m0_s[:, 0:S])

                # ctx^T (+ norm row 64) and state update share one PSUM bank
                cs = pp_cs.tile([128, 322], F32, tag="cs", name=f"cs{sc}_{h}")
                ps_c = cs[0:D + 2, 0:256]
                if sc > 0:
                    nc.tensor.matmul(ps_c, state_b[h], qfs[h][:, band],
                                     start=True, stop=False)
                    nc.tensor.matmul(ps_c, vp[0][h], atm[:, 0:256],
                                     start=False, stop=False)
                    nc.tensor.matmul(cs[0:D + 2, 128:256], vp[1][h],
                                     atm[:, 256:384], start=False, stop=True)
                else:
                    nc.tensor.matmul(ps_c, vp[0][h], atm[:, 0:256],
                                     start=True, stop=False)
                    nc.tensor.matmul(cs[0:D + 2, 128:256], vp[1][h],
                                     atm[:, 256:384], start=False, stop=True)

                ps_s = cs[:, 256:322]
                nc.tensor.matmul(ps_s, kfeat[0][h], vp[0][h], start=True, stop=False)
                nc.tensor.matmul(ps_s, kfeat[1][h], vp[1][h], start=False, stop=True)
                if sc == 0:
                    nc.vector.tensor_copy(state_f[h], ps_s)
                else:
                    nc.vector.tensor_add(state_f[h], state_f[h], ps_s)
                if sc < NSC - 1:
                    nc.gpsimd.tensor_copy(state_b[h], state_f[h])

                # norm row (+eps), reciprocal, bf16 cast (per head)
                nc.scalar.activation(
                    out=nrow[h], in_=cs[D:D + 1, 0:256],
                    func=mybir.ActivationFunctionType.Identity,
                    bias=eps_t[0:1, 0:1], scale=1.0)
                nc.vector.reciprocal(nrow[h], nrow[h])
                nc.gpsimd.tensor_copy(rnb[h], nrow[h])
                cs_h[h] = cs

            # PE-broadcast reciprocal rows -> normalized stacked ctx
            rbc = pp_mm.tile([128, 384], F32, tag="mm", name=f"rbc{sc}")
            nc.tensor.matmul(rbc[0:64, 0:256], ones1, rnb[0], start=True, stop=True)
            nc.tensor.matmul(rbc[64:128, 0:256], ones1, rnb[1], start=True, stop=True)
            rbc_sb = nrm_pool.tile([128, SC], F32, tag="rbc", name=f"rbcs{sc}")
            nc.scalar.copy(rbc_sb, rbc[:, 0:256])
            nc.vector.tensor_mul(ctxn[0:64, :], cs_h[0][0:D, 0:256],
                                 rbc_sb[0:64, :])
            nc.vector.tensor_mul(ctxn[64:128, :], cs_h[1][0:D, 0:256],
                                 rbc_sb[64:128, :])

            # per stripe: single stacked out-projection, copy, DMA
            for ci in range(2):
                ps_o = pp_big.tile([128, E], F32, tag="big", name=f"po{sc}_{ci}")
                nc.tensor.matmul(ps_o, ctxn[:, ci * S:(ci + 1) * S], w2p,
                                 start=True, stop=True)
                o_s = osb_pool.tile([128, E], BF16, tag="osb", name=f"os{sc}_{ci}")
                if ci == 0:
                    nc.scalar.copy(o_s, ps_o)
                else:
                    nc.vector.tensor_copy(o_s, ps_o)
                nc.sync.dma_start(out=out[sub[ci], :], in_=o_s)

    _split_multi_waits(nc)
    return nc


_PROGRAM = None


def _get_program():
    global _PROGRAM
    if _PROGRAM is None:
        _PROGRAM = build_program()
    return _PROGRAM


def _blocked(w):
    """[512, n] -> [128, 4*n] with kk-blocks of 128 contraction rows."""
    n = w.shape[1]
    return np.ascontiguousarray(
        w.reshape(4, 128, n).transpose(1, 0, 2).reshape(128, 4 * n))


def _make_in_maps(x, w_qkv, b_qkv, w_out):
    bf = ml_dtypes.bfloat16
    pos = np.arange(T, dtype=np.float32)
    ang = (math.pi / 2) * pos / T
    cosw = np.cos(ang).astype(np.float32)
    sinw = np.sin(ang).astype(np.float32)
    csrep = np.concatenate([
        np.broadcast_to(cosw[None, :], (D, T)),
        np.broadcast_to(sinw[None, :], (D, T)),
    ], 0)
    csswap = np.concatenate([
        np.broadcast_to(sinw[None, :], (D, T)),
        np.broadcast_to(cosw[None, :], (D, T)),
    ], 0)
    # cos/sin per stripe as [128, 16] per-partition columns
    spos = pos.reshape(8, 128)
    cscol = np.concatenate([
        np.cos((math.pi / 2) * spos / T),
        np.sin((math.pi / 2) * spos / T),
    ], 0).T.astype(np.float32)
    # causal mask with relative positional cos factor
    sp = np.arange(S)[:, None]
    tq = np.arange(SC)[None, :]
    maskc = ((sp <= tq) * np.cos((math.pi / 2) * (sp - tq) / T)).astype(np.float32)

    in_maps = []
    for i in range(8):
        b, g = divmod(i, 4)
        h0, h1 = 2 * g, 2 * g + 1
        wq = lambda h: w_qkv[h * D:(h + 1) * D]
        wk_ = lambda h: w_qkv[E + h * D:E + (h + 1) * D]
        wv_ = lambda h: w_qkv[2 * E + h * D:2 * E + (h + 1) * D]
        bq = lambda h: b_qkv[h * D:(h + 1) * D]
        bk = lambda h: b_qkv[E + h * D:E + (h + 1) * D]
        bv = lambda h: b_qkv[2 * E + h * D:2 * E + (h + 1) * D]
        hcols = np.r_[h0 * D:(h0 + 1) * D, h1 * D:(h1 + 1) * D]

        wq2 = np.concatenate([wq(h0), wq(h1)], 0).T      # [512, 128]
        wk2 = np.concatenate([wk_(h0), wk_(h1)], 0).T
        wv2 = np.concatenate([wv_(h0), wv_(h1)], 0).T
        wqkk = np.concatenate([_blocked(wq2), _blocked(wk2)], 1)

        bcol = np.stack([
            np.concatenate([bq(h0), bq(h1)]),
            np.concatenate([bk(h0), bk(h1)]),
            np.concatenate([bv(h0), bv(h1)]),
            np.zeros(128, np.float32),
        ], 1)  # [128, 4]
        combo1 = np.concatenate([np.eye(128, dtype=np.float32), bcol, cscol], 1)

        w2pack = w_out[:, hcols].T                        # [128, 512]
        combo2 = np.concatenate([
            _blocked(wv2), csrep, csswap, maskc, w2pack], 1)

        in_maps.append({
            "xtp": np.ascontiguousarray(
                x[b].T.reshape(4, 128, 2, 512).transpose(1, 2, 0, 3)
                .reshape(128, 4096)).astype(bf),
            "wqkk": wqkk.astype(bf),
            "combo1": combo1.astype(bf),
            "combo2": combo2.astype(bf),
        })
    return in_maps


def run(inputs, trace=False):
    x = np.asarray(inputs["x"], dtype=np.float32)
    w_qkv = np.asarray(inputs["w_qkv"], dtype=np.float32)
    b_qkv = np.asarray(inputs["b_qkv"], dtype=np.float32)
    w_out = np.asarray(inputs["w_out"], dtype=np.float32)
    b_out = np.asarray(inputs["b_out"], dtype=np.float32)

    nc = _get_program()
    in_maps = _make_in_maps(x, w_qkv, b_qkv, w_out)
    res = run_bass_kernel_spmd(nc, in_maps, list(range(8)), trace=trace)

    out = np.empty((B, T, E), dtype=np.float32)
    for b in range(B):
        acc = res.results[4 * b]["out"].astype(np.float32)
        for g in range(1, 4):
            acc = acc + res.results[4 * b + g]["out"]
        out[b] = acc + b_out[None, :]
    return out, res


def kernel(**inputs) -> np.ndarray:
    out, _ = run(inputs, trace=False)
    return out
